# revision 1
# baseline (speedup 1.0000x reference)
"""LConv (7x7 position-linear conv) Trainium2 Bass kernel.

Full inputs in, full output out. Sharding: data-parallel over batch,
16 images -> 8 NeuronCores (2 images/core). abc/bias replicated.

Math (K=7, S=1, P=3, D=1, G=1, C=COUT=128):
  out[o,i,j] = sum_c sum_{t=1..7} P_t[c,o]*W1[c,i+t,j] + B[c,o]*W2[c,i+t,j]
             + bias[o]
  W1 = 7-wide box sum along W of padded x; W2 = position-ramp sum along W.
  Both are computed from running sums (cumsum) along each padded row:
  rows are stored 122 wide = 7 lead zeros + 112 data + 3 trail zeros, so
  cumsum(stream shifted by 7) - cumsum(stream) telescopes exactly to the
  7-tap box (the 7 lead zeros kill the stream-prefix offset).
  P_t = (t-4)*A + C ; A=abc[0:128], B=abc[128:256], C=abc[256:384].
"""

import numpy as np

import concourse.bacc as bacc
import concourse.mybir as mybir
from concourse import tile
from concourse.bass_utils import run_bass_kernel_spmd

F32 = mybir.dt.float32
BF16 = mybir.dt.bfloat16
I32 = mybir.dt.int32
AF = mybir.ActivationFunctionType
ALU = mybir.AluOpType

USE_CUSTOM_OPA = True   # fused scan-diff custom DVE op
USE_PAIR_SUMS = True    # gpsimd pair-sums -> 11 matmuls/tile instead of 14

B_TOT = 16
N_CORES = 8
B_PER = B_TOT // N_CORES
CIN = 128
COUT = 128
H = W = 112
PW2 = 122                 # row layout: 7 lead zeros + 112 data + 3 trail
LEAD = 7
PH = 119                  # padded row count: 4 lead + 112 + 3 trail
RLEAD = 4
ROWS_PER_SLAB = 16
N_SLABS = H // ROWS_PER_SLAB
SLAB_FREE = ROWS_PER_SLAB * PW2        # 1952
DSTREAM = SLAB_FREE - 7                # 1945
WFULL = PH * PW2                       # 14518 (stream layout, bf16)
OUT_TILE_ROWS = 4
N_OUT_TILES = H // OUT_TILE_ROWS
OUT_TILE_FREE = OUT_TILE_ROWS * W      # 448

_CACHE = {}


def _register_opa():
    from concourse.dve_spec import Spec, Src0, Src1, scan, AluOp, lower
    import concourse.dve_ops as dve_ops
    from concourse.dve_uop import DveOpSpec

    if any(op.name == "BOXDIFF7" for op in dve_ops.OPS):
        return next(op for op in dve_ops.OPS if op.name == "BOXDIFF7")
    spec = Spec(
        body=scan(AluOp.ADD, Src0) - scan(AluOp.ADD, Src1),
        reference=lambda in0, in1: (
            np.cumsum(in0, axis=-1) - np.cumsum(in1, axis=-1)
        ),
    )
    row = dve_ops._CUSTOM_DVE_ROW_BASE + len(dve_ops.OPS)
    shas = {}
    for ver in ("v3", "v4"):
        s = DveOpSpec(
            name="BOXDIFF7", opcode=row, uops=lower(spec, ver=ver), rd1_en=True
        )
        shas[ver] = s.sha(ver)
    op = dve_ops.DveOp("BOXDIFF7", spec, subdim=False, uops_sha=shas)
    dve_ops.OPS.append(op)
    dve_ops._SUB_OPCODE_FOR_NAME[op.name] = row
    dve_ops.CUSTOM_DVE_SPECS[op.name] = op.spec
    return op


def _build():
    nc = bacc.Bacc("TRN2", target_bir_lowering=False, debug=False)
    opa = _register_opa() if USE_CUSTOM_OPA else None

    t_x = nc.dram_tensor("xs", [B_PER, CIN, H, W], F32, kind="ExternalInput")
    t_pw = nc.dram_tensor("pw", [7, CIN, COUT], F32, kind="ExternalInput")
    t_bw = nc.dram_tensor("bw", [CIN, COUT], F32, kind="ExternalInput")
    t_bias = nc.dram_tensor("bias", [COUT, 1], F32, kind="ExternalInput")
    t_out = nc.dram_tensor("out", [B_PER, COUT, H, W], F32, kind="ExternalOutput")

    with tile.TileContext(nc) as tc:
        with (
            tc.tile_pool(name="const", bufs=1) as cpool,
            tc.tile_pool(name="wfull", bufs=1) as wpool,
            tc.tile_pool(name="slab", bufs=2) as spool,
            tc.tile_pool(name="outs", bufs=3) as opool,
            tc.tile_pool(name="ps", bufs=4, space="PSUM") as ppool,
        ):
            # ---- constants ----
            pw_f = cpool.tile([CIN, 7 * COUT], F32, tag="pwf")
            nc.sync.dma_start(
                pw_f[:].rearrange("c (t o) -> c t o", t=7),
                t_pw[:].transpose([1, 0, 2]),
            )
            pw = cpool.tile([CIN, 7 * COUT], BF16, tag="pwb")
            nc.vector.tensor_copy(pw[:], pw_f[:])
            bw_f = cpool.tile([CIN, COUT], F32, tag="bwf")
            nc.sync.dma_start(bw_f[:], t_bw[:])
            bw = cpool.tile([CIN, COUT], BF16, tag="bwb")
            nc.vector.tensor_copy(bw[:], bw_f[:])
            bias_sb = cpool.tile([COUT, 1], F32, tag="bias")
            nc.sync.dma_start(bias_sb[:], t_bias[:])

            # col-position map for the ramp: value (p-3) at col p of the
            # 122-grid == (data col + 4), matching the recenter term (j+4).
            jmap_i = cpool.tile([128, SLAB_FREE], I32, tag="jmapi")
            nc.gpsimd.iota(
                jmap_i[:], pattern=[[0, ROWS_PER_SLAB], [1, PW2]],
                base=-3, channel_multiplier=0,
            )
            jmap = cpool.tile([128, SLAB_FREE], F32, tag="jmap")
            nc.vector.tensor_copy(jmap[:], jmap_i[:])
            jp4_i = cpool.tile([128, ROWS_PER_SLAB * W], I32, tag="jp4i")
            nc.gpsimd.iota(
                jp4_i[:], pattern=[[0, ROWS_PER_SLAB], [1, W]],
                base=4, channel_multiplier=0,
            )
            jp4 = cpool.tile([128, ROWS_PER_SLAB * W], F32, tag="jp4")
            nc.vector.tensor_copy(jp4[:], jp4_i[:])

            # ---- full-image W1/W2 streams (bf16, PH x PW2 layout) ----
            w1 = wpool.tile([CIN, WFULL], BF16, tag="w1")
            w2 = wpool.tile([CIN, WFULL], BF16, tag="w2")
            nc.vector.memset(w1[:, : RLEAD * PW2], 0.0)
            nc.vector.memset(w1[:, (RLEAD + H) * PW2 :], 0.0)
            nc.vector.memset(w2[:, : RLEAD * PW2], 0.0)
            nc.vector.memset(w2[:, (RLEAD + H) * PW2 :], 0.0)
            if USE_PAIR_SUMS:
                w2p = wpool.tile([CIN, WFULL], BF16, tag="w2p")
                nc.vector.memset(w2p[:, : RLEAD * PW2], 0.0)
                nc.vector.memset(w2p[:, (RLEAD + H - 1) * PW2 :], 0.0)

            xp_bufs = []
            for i in range(2):
                xpb = spool.tile([CIN, SLAB_FREE], F32, tag=f"xp{i}")
                nc.vector.memset(xpb[:], 0.0)
                xp_bufs.append(xpb)

            def row_view(buf, r0, nrows=ROWS_PER_SLAB):
                # strided (nrows,112) view at data cols of the 122-grid
                base = (RLEAD + r0) * PW2
                return buf[:, base : base + nrows * PW2].rearrange(
                    "c (r q) -> c r q", q=PW2
                )[:, :, 3:115]

            for b in range(B_PER):
                # ---------- stage 1: W-direction filters ----------
                for s in range(N_SLABS):
                    r0 = s * ROWS_PER_SLAB
                    xp = xp_bufs[s % 2]
                    nc.sync.dma_start(
                        xp[:].rearrange("c (r q) -> c r q", r=ROWS_PER_SLAB)[
                            :, :, LEAD : LEAD + W
                        ],
                        t_x[b, :, r0 : r0 + ROWS_PER_SLAB, :],
                    )
                    w1s = w1[:, (RLEAD + r0) * PW2 : (RLEAD + r0) * PW2 + DSTREAM]
                    w2s = w2[:, (RLEAD + r0) * PW2 : (RLEAD + r0) * PW2 + DSTREAM]
                    if USE_CUSTOM_OPA:
                        d1 = spool.tile([CIN, SLAB_FREE], F32, tag="d1")
                        nc.vector._custom_dve(
                            opa, out=d1[:, :DSTREAM], in0=xp[:, 7:], in1=xp[:, :DSTREAM]
                        )
                        nc.scalar.copy(w1s, d1[:, :DSTREAM])
                        nc.vector.tensor_tensor(xp[:], xp[:], jmap[:], op=ALU.mult)
                        rawd = spool.tile([CIN, SLAB_FREE], F32, tag="rawd")
                        nc.vector._custom_dve(
                            opa, out=rawd[:, :DSTREAM], in0=xp[:, 7:], in1=xp[:, :DSTREAM]
                        )
                    else:
                        c1 = spool.tile([CIN, SLAB_FREE], F32, tag="c1")
                        nc.vector.tensor_tensor_scan(
                            c1[:], xp[:], xp[:], 0.0, op0=ALU.add, op1=ALU.bypass
                        )
                        d1 = spool.tile([CIN, SLAB_FREE], F32, tag="d1")
                        nc.vector.tensor_tensor(
                            d1[:, :DSTREAM], c1[:, 7:], c1[:, :DSTREAM], op=ALU.subtract
                        )
                        nc.scalar.copy(w1s, d1[:, :DSTREAM])
                        nc.vector.tensor_tensor(xp[:], xp[:], jmap[:], op=ALU.mult)
                        cj = spool.tile([CIN, SLAB_FREE], F32, tag="cj")
                        nc.vector.tensor_tensor_scan(
                            cj[:], xp[:], xp[:], 0.0, op0=ALU.add, op1=ALU.bypass
                        )
                        rawd = spool.tile([CIN, SLAB_FREE], F32, tag="rawd")
                        nc.vector.tensor_tensor(
                            rawd[:, :DSTREAM], cj[:, 7:], cj[:, :DSTREAM], op=ALU.subtract
                        )
                    # w2b = (j+4) * W1  (gpsimd, fp32, strided d1 view)
                    d1v = d1[:].rearrange("c (r q) -> c r q", q=PW2)[:, :, 3:115]
                    w2b = spool.tile([CIN, ROWS_PER_SLAB * W], F32, tag="w2b")
                    nc.gpsimd.tensor_tensor(
                        w2b[:].rearrange("c (r j) -> c r j", r=ROWS_PER_SLAB),
                        jp4[:].rearrange("c (r j) -> c r j", r=ROWS_PER_SLAB),
                        d1v,
                        op=ALU.mult,
                    )
                    # w2 = rawd - w2b (bf16 cast on write; values are small)
                    rawv = rawd[:].rearrange("c (r q) -> c r q", q=PW2)[:, :, 3:115]
                    nc.vector.tensor_tensor(
                        row_view(w2, r0),
                        rawv,
                        w2b[:].rearrange("c (r j) -> c r j", r=ROWS_PER_SLAB),
                        op=ALU.subtract,
                    )
                    if USE_PAIR_SUMS and s > 0:
                        # pair rows r-1..: w2p[r] = w2[r] + w2[r+1]
                        pr0 = (s - 1) * ROWS_PER_SLAB
                        nc.gpsimd.tensor_tensor(
                            row_view(w2p, pr0),
                            row_view(w2, pr0),
                            row_view(w2, pr0 + 1),
                            op=ALU.add,
                        )
                if USE_PAIR_SUMS:
                    # last slab's pairs + the pad-row boundary pairs
                    pr0 = (N_SLABS - 1) * ROWS_PER_SLAB
                    nc.gpsimd.tensor_tensor(
                        row_view(w2p, pr0, ROWS_PER_SLAB - 1),
                        row_view(w2, pr0, ROWS_PER_SLAB - 1),
                        row_view(w2, pr0 + 1, ROWS_PER_SLAB - 1),
                        op=ALU.add,
                    )
                    # rows -4..-1 (lead pad rows -4..-2 pair into data row 0)
                    nc.gpsimd.tensor_tensor(
                        row_view(w2p, -4, 4),
                        row_view(w2, -4, 4),
                        row_view(w2, -3, 4),
                        op=ALU.add,
                    )
                    # trailing: row H-1 pairs with pad row H (zero) etc
                    nc.gpsimd.tensor_tensor(
                        row_view(w2p, H - 1, 3),
                        row_view(w2, H - 1, 3),
                        row_view(w2, H, 3),
                        op=ALU.add,
                    )

                # ---------- stage 2: PE folds over H-shifts ----------
                for it in range(N_OUT_TILES):
                    i0 = it * OUT_TILE_ROWS
                    acc = ppool.tile([COUT, OUT_TILE_FREE], F32, tag="acc")

                    def rhs(buf, trow):
                        base = (i0 + trow) * PW2
                        return buf[:, base : base + OUT_TILE_ROWS * PW2].rearrange(
                            "c (r q) -> c r q", q=PW2
                        )[:, :, 3:115]

                    first = True
                    for t in range(1, 8):
                        nc.tensor.matmul(
                            acc[:],
                            pw[:, (t - 1) * COUT : t * COUT],
                            rhs(w1, t),
                            start=first,
                            stop=False,
                        )
                        first = False
                    if USE_PAIR_SUMS:
                        # box7(w2) = w2p[1] + w2p[3] + w2p[5] + w2[7]
                        for t in (1, 3, 5):
                            nc.tensor.matmul(
                                acc[:], bw[:], rhs(w2p, t), start=False, stop=False
                            )
                        nc.tensor.matmul(
                            acc[:], bw[:], rhs(w2, 7), start=False, stop=True
                        )
                    else:
                        for t in range(1, 8):
                            nc.tensor.matmul(
                                acc[:], bw[:], rhs(w2, t), start=False, stop=(t == 7)
                            )
                    ot = opool.tile([COUT, OUT_TILE_FREE], F32, tag="ot")
                    nc.scalar.activation(
                        ot[:], acc[:], AF.Identity, bias=bias_sb[:], scale=1.0
                    )
                    nc.sync.dma_start(
                        t_out[b, :, i0 : i0 + OUT_TILE_ROWS, :].rearrange(
                            "o r j -> o (r j)"
                        ),
                        ot[:],
                    )

    nc.compile()
    return nc


def kernel(x: np.ndarray, abc: np.ndarray, bias: np.ndarray) -> np.ndarray:
    x = np.ascontiguousarray(x, dtype=np.float32)
    abc = np.asarray(abc, dtype=np.float32)
    bias = np.asarray(bias, dtype=np.float32)

    if "nc" not in _CACHE:
        _CACHE["nc"] = _build()
    nc = _CACHE["nc"]

    A, Bm, Cc = abc[0:128], abc[128:256], abc[256:384]
    pw = np.stack([(t - 4.0) * A + Cc for t in range(1, 8)]).astype(np.float32)
    in_maps = []
    for c in range(N_CORES):
        in_maps.append(
            {
                "xs": x[c * B_PER : (c + 1) * B_PER],
                "pw": pw,
                "bw": np.ascontiguousarray(Bm),
                "bias": np.ascontiguousarray(bias.reshape(COUT, 1)),
            }
        )
    res = run_bass_kernel_spmd(nc, in_maps, list(range(N_CORES)))
    out = np.concatenate([res.results[c]["out"] for c in range(N_CORES)], axis=0)
    return out.astype(np.float32)


if __name__ == "__main__":
    rng = np.random.default_rng(0)
    x = rng.standard_normal((16, 128, 112, 112), dtype=np.float32)
    abc = (rng.standard_normal((384, 128)) * 0.05).astype(np.float32)
    bias = (rng.standard_normal((128,)) * 0.05).astype(np.float32)
    out = kernel(x=x, abc=abc, bias=bias)
    print(out.shape, out.dtype)

